# revision 60
# baseline (speedup 1.0000x reference)
"""Trainium2 Bass kernel for nn_MACBlock (segmented attention + GEGLU FFN).

Sharding: 8 cores = 2 batches x 4 segments of 512 queries. The segment mask
makes attention block-diagonal (plus a 32-token always-visible prefix derived
from pooled memory + persistent memory), so each core is fully independent:
no collectives.

Layout: activations are kept feature-major (x^T [dim, tokens]) on-chip, so
every matmul contraction dim lands on partitions with zero transposes.
All matmuls run with bf16 operands (fp32 PSUM accumulation); weights are
shipped bf16 in partition-major DRAM layouts so each dma_start moves large
per-partition-contiguous lines. The softmax denominator comes from a column
of ones appended to V (one extra PSUM row), and normalization is a bf16
reciprocal of that row broadcast through the PE.
"""

import sys

if "/opt/trn_rl_repo" not in sys.path:
    sys.path.insert(0, "/opt/trn_rl_repo")

import numpy as np

B, N, DIM = 2, 2048, 1024
HEADS, DH = 16, 64
DHP = DH + 1            # +1 denominator column
SEG = 512
NPM = NM = 16
PFX = NPM + NM          # 32 prefix keys
DFF = 2730
MFF = 22                # padded dff chunks
DFFP = MFF * 128        # 2816
KO = 8                  # 1024 / 128
P = 128
NCORES = 8
EPS = 1.1920929e-07

_CACHE = {}


def build_nc(reps=1):
    import concourse.bass as bass
    from concourse import bacc
    import concourse.tile as tile
    import concourse.mybir as mybir

    f32 = mybir.dt.float32
    bf = mybir.dt.bfloat16
    f8 = mybir.dt.float8e4
    DR = mybir.MatmulPerfMode.DoubleRow
    AF = mybir.ActivationFunctionType
    OP = mybir.AluOpType
    AX = mybir.AxisListType

    nc = bacc.Bacc("TRN2", target_bir_lowering=False, debug=False)

    dp = nc.declare_dram_parameter
    xT_d = dp("xT", [P, KO, SEG], f32, isOutput=False)
    mo_d = dp("mo", [P, 16, DIM], bf, isOutput=False)
    cq_d = dp("cq", [P, SEG], bf, isOutput=False)
    sq_d = dp("sq", [P, SEG], bf, isOutput=False)
    ck_d = dp("ck", [P, SEG], bf, isOutput=False)
    sk_d = dp("sk", [P, SEG], bf, isOutput=False)
    mask_d = dp("maskD", [P, P], bf, isOutput=False)
    rmat_d = dp("rmat", [P, P], bf, isOutput=False)
    ones_d = dp("ones", [P, P], bf, isOutput=False)
    qkw_d = dp("qkw", [P, 16, KO, P], bf, isOutput=False)
    kvw_d = dp("kvw", [P, 2, KO, DIM], bf, isOutput=False)
    outw_d = dp("outw", [P, KO, KO, P], bf, isOutput=False)
    w1_d = dp("w1", [P, MFF, 2, 4, 2, P], f8, isOutput=False)
    w2_d = dp("w2", [KO, P, 11, 2, P], f8, isOutput=False)
    mtw_d = dp("mtw", [P, KO, DIM], bf, isOutput=False)
    pmv_d = dp("pmv", [HEADS, NPM, DHP], bf, isOutput=False)
    pmk_d = dp("pmk", [HEADS, DH, NPM], bf, isOutput=False)
    selv_d = dp("selv", [P, HEADS, HEADS], bf, isOutput=False)
    selb_d = dp("selb", [HEADS, HEADS, DH], bf, isOutput=False)
    b1a_d = dp("b1a", [P, MFF], f32, isOutput=False)
    b1g_d = dp("b1g", [P, MFF], f32, isOutput=False)
    anw_d = dp("anw", [P, KO], f32, isOutput=False)
    fnw_d = dp("fnw", [P, KO], f32, isOutput=False)
    mpnw_d = dp("mpnw", [1, DIM], f32, isOutput=False)
    yT_d = dp("yT", [DIM, SEG], f32, isOutput=True)

    def _emit(nc):
      with tile.TileContext(nc) as tc, \
            nc.allow_low_precision(reason="bf16 matmul datapath"):
        from contextlib import ExitStack
        ctx = ExitStack()
        with ctx:
            persist = ctx.enter_context(tc.tile_pool(name="persist", bufs=1))
            wpool = ctx.enter_context(tc.tile_pool(name="wpool", bufs=3))
            kvres = ctx.enter_context(tc.tile_pool(name="kvres", bufs=1))
            w2pool = ctx.enter_context(tc.tile_pool(name="w2pool", bufs=2))
            mopool = ctx.enter_context(tc.tile_pool(name="mopool", bufs=2))
            rot = ctx.enter_context(tc.tile_pool(name="rot", bufs=2))
            epool = ctx.enter_context(tc.tile_pool(name="epool", bufs=8))
            eppool = ctx.enter_context(tc.tile_pool(name="eppool", bufs=2))
            pa = ctx.enter_context(tc.tile_pool(name="pa", bufs=3, space="PSUM"))
            psc = ctx.enter_context(tc.tile_pool(name="psc", bufs=2, space="PSUM"))
            pso = ctx.enter_context(tc.tile_pool(name="pso", bufs=2, space="PSUM"))
            dnp = ctx.enter_context(tc.tile_pool(name="dnp", bufs=1, space="PSUM"))

            cnt = [0]

            def pa_t():
                cnt[0] += 1
                return pa.tile([P, SEG], f32, tag="ps", name=f"pa{cnt[0]}")

            def psc_t():
                cnt[0] += 1
                return psc.tile([P, SEG], f32, tag="sc", name=f"sc{cnt[0]}")

            def pso_t():
                cnt[0] += 1
                return pso.tile([P, SEG], f32, tag="o", name=f"o{cnt[0]}")

            # ---------------- persistent SBUF tensors ----------------
            xT = persist.tile([P, KO, SEG], f32, tag="xT")       # x^T, later x1^T
            xnT = persist.tile([P, KO, SEG], bf, tag="xnT")      # xn^T (bf16)
            qT = persist.tile([P, KO, SEG], bf, tag="qT")        # roped,scaled q^T
            kT = persist.tile([P, KO, SEG], bf, tag="kT")        # roped k^T
            vA = persist.tile([P, 4, HEADS, DHP], bf, tag="vA")  # v key-major +ones
            # prefix v rows (+ones col), head pair j: even head at rows 0:32,
            # odd head at rows 32:64 (matching exp-pair partition layout)
            vP2 = persist.tile([2 * PFX, KO, DHP], bf, tag="vP2")
            kP = persist.tile([P, HEADS, PFX], bf, tag="kP")     # prefix k^T @64*(h%2)
            oA = persist.tile([P, KO, SEG], bf, tag="oA")        # attn o^T
            cq = persist.tile([P, SEG], bf, tag="cq")
            sq_ = persist.tile([P, SEG], bf, tag="sq")
            ck = persist.tile([P, SEG], bf, tag="ck")
            sk = persist.tile([P, SEG], bf, tag="sk")
            maskD = persist.tile([P, P], bf, tag="maskD")
            rmat = persist.tile([P, P], bf, tag="rmat")
            b1a = persist.tile([P, MFF], f32, tag="b1a")
            b1g = persist.tile([P, MFF], f32, tag="b1g")
            anw = persist.tile([P, KO], f32, tag="anw")
            fnw = persist.tile([P, KO], f32, tag="fnw")
            mpnw = persist.tile([1, DIM], f32, tag="mpnw")
            ones128 = persist.tile([P, 1], bf, tag="o128")       # lhsT K=128,M=1
            ones1x128 = persist.tile([1, P], bf, tag="o1x128")   # lhsT K=1,M=128
            ones16 = persist.tile([1, 16], bf, tag="o16")
            ones11 = persist.tile([1, 1], bf, tag="o11")
            pooledT = persist.tile([P, KO], bf, tag="pooledT")
            memtokT = persist.tile([P, KO], bf, tag="memtokT")
            mrow = persist.tile([1, 2 * DIM], f32, tag="mrow")
            brow = persist.tile([1, 2 * DIM], bf, tag="brow")    # bf16 row scratch
            rrow = persist.tile([1, SEG], bf, tag="rrow")        # bf16 rstd row
            rcp = persist.tile([16, SEG], bf, tag="rcp")         # 1/denom
            onesDH = persist.tile([DH, SEG], bf, tag="onesDH")   # TT-copy helper
            zrow64 = persist.tile([DHP, SEG], bf, tag="zrow64")  # TT-copy helper
            selv = persist.tile([P, HEADS, HEADS], bf, tag="selv")
            selb = persist.tile([HEADS, HEADS, DH], bf, tag="selb")
            inv256c = persist.tile([P, 1], f32, tag="inv256c")
            epsc = persist.tile([P, 1], f32, tag="epsc")
            zeroc = persist.tile([P, 1], f32, tag="zeroc")

            dma = nc.sync.dma_start
            # Critical-path loads first: the attn rmsnorm needs xT/ones/anw,
            # the first q/k projections need qkw chunk 0 + rope tables.
            dma(out=xT, in_=xT_d[:])
            dma(out=ones128, in_=ones_d[:, 0:1])
            dma(out=ones1x128, in_=ones_d[0:1, :])
            dma(out=anw, in_=anw_d[:])
            wt0 = wpool.tile([P, 4, KO, P], bf, tag="w", name="wt0")
            dma(out=wt0, in_=qkw_d[:, 0:4])
            mot0 = mopool.tile([P, 1, DIM], bf, tag="mo", name="mot0")
            dma(out=mot0, in_=mo_d[:, 0:1])
            dma(out=cq, in_=cq_d[:])
            dma(out=sq_, in_=sq_d[:])
            dma(out=rmat, in_=rmat_d[:])
            dma(out=ck, in_=ck_d[:])
            dma(out=sk, in_=sk_d[:])
            dma(out=maskD, in_=mask_d[:])
            dma(out=b1a, in_=b1a_d[:])
            dma(out=b1g, in_=b1g_d[:])
            dma(out=fnw, in_=fnw_d[:])
            dma(out=mpnw, in_=mpnw_d[:])
            dma(out=ones16, in_=ones_d[0:1, 0:16])
            dma(out=ones11, in_=ones_d[0:1, 0:1])
            dma(out=selv, in_=selv_d[:])
            dma(out=selb, in_=selb_d[:])
            nc.vector.memset(inv256c, 1.0 / 256.0)
            nc.vector.memset(epsc, EPS)
            nc.vector.memset(zeroc, 0.0)
            nc.vector.memset(vA[:, :, :, DH:DHP], 1.0)
            nc.vector.memset(vP2[:, :, DH:DHP], 1.0)
            nc.vector.memset(onesDH, 1.0)
            nc.vector.memset(zrow64, 0.0)
            for h in range(HEADS):
                hb = DH * (h % 2)
                j, b32 = h // 2, PFX * (h % 2)
                dma(out=kP[hb:hb + DH, h, NPM:PFX], in_=pmk_d[h])
                dma(out=vP2[b32 + NPM:b32 + PFX, j, :], in_=pmv_d[h])

            if True:
              kvw = kvres.tile([P, 2, KO, DIM], bf, tag="kv")

              mm = nc.tensor.matmul

              def rmsnorm_into(dst_fn, src, w_sb):
                  """dst(ko) = src[:,ko,:] * w[:,ko] * rsqrt(mean_dim(src^2)+eps)"""
                  ss = pso_t()  # [1,512] slice used
                  for ko in range(KO):
                      sq_t = rot.tile([P, SEG], bf, tag="sqt")
                      nc.vector.tensor_mul(sq_t, src[:, ko, :],
                                           src[:, ko, :])
                      mm(ss[0:1, :], ones128, sq_t,
                         start=(ko == 0), stop=(ko == KO - 1))
                  # rstd = exp(-0.5*ln(ms+eps)) — pure ACT, no DVE reciprocal
                  nc.scalar.activation(mrow[0:1, 0:SEG], ss[0:1, :], AF.Ln,
                                       bias=epsc[0:1], scale=1.0 / DIM)
                  nc.scalar.activation(rrow, mrow[0:1, 0:SEG], AF.Exp,
                                       bias=zeroc[0:1], scale=-0.5)
                  bc = pso_t()  # broadcast rstd over 128 partitions
                  mm(bc, ones1x128, rrow, start=True, stop=True)
                  for ko in range(KO):
                      nc.vector.scalar_tensor_tensor(
                          out=dst_fn(ko), in0=src[:, ko, :],
                          scalar=w_sb[:, ko:ko + 1], in1=bc,
                          op0=OP.mult, op1=OP.mult)

              # ---------------- mem_out mean chunks 0-3 first: these depend
              # only on their own DMAs, filling PE while xT/qkw land --------
              mean_ps = [psc_t(), psc_t()]   # two [1,512] accumulators (slices)
              mh = mot0
              for m in range(4):
                  if m > 0:
                      mh = mopool.tile([P, 1, DIM], bf, tag="mo")
                      dma(out=mh, in_=mo_d[:, m:m + 1])
                  for half in range(2):
                      mm(mean_ps[half][0:1, :], ones128,
                         mh[:, 0, half * SEG:(half + 1) * SEG],
                         start=(m == 0), stop=False)

              # ---------------- attn rmsnorm ----------------
              rmsnorm_into(lambda ko: xnT[:, ko, :], xT, anw)

              # ---------------- q/k projections + rope, interleaved with
              # ---------------- mem_out mean accumulation (chunks 4-15) ----
              wt = wt0
              for m in range(16):
                  if m % 4 == 0 and m > 0:
                      wt = wpool.tile([P, 4, KO, P], bf, tag="w")
                      dma(out=wt, in_=qkw_d[:, m:m + 4])
                  if m == 1:   # v-weights: needed right after this loop
                      dma(out=kvw[:, 1], in_=kvw_d[:, 1])
                  if m == 9:   # k-weights: needed by the mem chain
                      dma(out=kvw[:, 0], in_=kvw_d[:, 0])
                  if m >= 4:
                      mot = mopool.tile([P, 1, DIM], bf, tag="mo")
                      dma(out=mot, in_=mo_d[:, m:m + 1])
                  ps = pa_t()
                  for ko in range(KO):
                      mm(ps, wt[:, m % 4, ko], xnT[:, ko, :],
                         start=(ko == 0), stop=(ko == KO - 1))
                  is_q = m < 8
                  c_t, s_t = (cq, sq_) if is_q else (ck, sk)
                  dst = qT if is_q else kT
                  ko_out = m % 8
                  qraw = rot.tile([P, SEG], bf, tag="ropeA")
                  nc.scalar.copy(qraw, ps)
                  rps = pa_t()
                  mm(rps, rmat, qraw, start=True, stop=True)
                  At = rot.tile([P, SEG], bf, tag="ropeB")
                  nc.vector.tensor_mul(At, ps, c_t)
                  Bt = rot.tile([P, SEG], bf, tag="ropeC")
                  nc.vector.tensor_mul(Bt, rps, s_t)
                  nc.vector.tensor_add(dst[:, ko_out, :], At, Bt)
                  # mem_out mean accumulation (chunk m; 0-3 hoisted above)
                  if m >= 4:
                      for half in range(2):
                          mm(mean_ps[half][0:1, :], ones128,
                             mot[:, 0, half * SEG:(half + 1) * SEG],
                             start=False, stop=(m == 15))

              # ---------------- v projection (token-major) ----------------
              for half in range(2):
                  for tc_ in range(4):
                      ps = pa_t()
                      for ko in range(KO):
                          mm(ps, xnT[:, ko, tc_ * P:(tc_ + 1) * P],
                             kvw[:, 1, ko, half * SEG:(half + 1) * SEG],
                             start=(ko == 0), stop=(ko == KO - 1))
                      nc.vector.tensor_copy(
                          out=vA[:, tc_, half * 8:(half + 1) * 8, 0:DH],
                          in_=ps.rearrange("p (h d) -> p h d", d=DH))

              # ---------------- memory-context chain ----------------
              pooled_raw = mrow[:, 0:DIM]
              for half in range(2):
                  nc.scalar.activation(pooled_raw[:, half * SEG:(half + 1) * SEG],
                                       mean_ps[half][0:1, :], AF.Copy,
                                       scale=1.0 / N)
              sqr = mrow[:, DIM:2 * DIM]
              nc.vector.tensor_mul(sqr, pooled_raw, pooled_raw)
              nc.vector.reduce_sum(sqr[:, 0:1], sqr, axis=AX.X)
              nc.scalar.activation(sqr[:, 1:2], sqr[:, 0:1], AF.Ln,
                                   bias=epsc[0:1], scale=1.0 / DIM)
              nc.scalar.activation(sqr[:, 2:3], sqr[:, 1:2], AF.Exp,
                                   bias=zeroc[0:1], scale=-0.5)
              pooled = brow[:, 0:DIM]
              nc.vector.scalar_tensor_tensor(out=pooled, in0=pooled_raw,
                                             scalar=sqr[:, 2:3], in1=mpnw,
                                             op0=OP.mult, op1=OP.mult)
              # pooled^T via K=1 transpose matmuls
              pT = pa_t()
              for ko in range(KO):
                  mm(pT[:, ko:ko + 1], pooled[0:1, ko * P:(ko + 1) * P],
                     ones11, start=True, stop=True, skip_group_check=True)
              nc.vector.tensor_copy(out=pooledT, in_=pT[:, 0:KO])
              # mem_tok row = pooled @ to_mem_tokens_w
              mt_ps = [psc_t(), psc_t()]
              for j in range(2):
                  mtw_t = w2pool.tile([P, 4, DIM], bf, tag="mtw")
                  dma(out=mtw_t, in_=mtw_d[:, 4 * j:4 * j + 4])
                  for k2 in range(4):
                      ko = 4 * j + k2
                      for half in range(2):
                          mm(mt_ps[half][0:1, :], pooledT[:, ko:ko + 1],
                             mtw_t[:, k2, half * SEG:(half + 1) * SEG],
                             start=(ko == 0), stop=(ko == KO - 1))
              memtok = brow[:, DIM:2 * DIM]
              for half in range(2):
                  nc.scalar.activation(memtok[:, half * SEG:(half + 1) * SEG],
                                       mt_ps[half][0:1, :], AF.Copy)
              mT = pa_t()
              for ko in range(KO):
                  mm(mT[:, ko:ko + 1], memtok[0:1, ko * P:(ko + 1) * P],
                     ones11, start=True, stop=True, skip_group_check=True)
              nc.vector.tensor_copy(out=memtokT, in_=mT[:, 0:KO])
              # k_c / v_c rows = mem_tok @ Wk / Wv
              kcvc = []
              for c in range(2):
                  r_ps = [psc_t(), psc_t()]
                  for ko in range(KO):
                      for half in range(2):
                          mm(r_ps[half][0:1, :], memtokT[:, ko:ko + 1],
                             kvw[:, c, ko, half * SEG:(half + 1) * SEG],
                             start=(ko == 0), stop=(ko == KO - 1))
                  row = brow[:, 0:DIM] if c == 0 else brow[:, DIM:2 * DIM]
                  for half in range(2):
                      nc.scalar.activation(row[:, half * SEG:(half + 1) * SEG],
                                           r_ps[half][0:1, :], AF.Copy)
                  kcvc.append(row)
              kc_row, vc_row = kcvc
              # k_extra^T into kP (16 identical columns per head)
              for j in range(KO):  # 2 heads per chunk
                  kx = pa_t()
                  mm(kx[:, 0:16], kc_row[0:1, j * P:(j + 1) * P],
                     ones16, start=True, stop=True, skip_group_check=True)
                  nc.vector.tensor_copy(out=kP[0:DH, 2 * j, 0:NPM],
                                        in_=kx[0:DH, 0:16])
                  nc.vector.tensor_copy(out=kP[DH:P, 2 * j + 1, 0:NPM],
                                        in_=kx[DH:P, 0:16])
              # v_extra rows into vP2 (16 identical rows per head)
              for half in range(2):
                  vx = pa_t()
                  mm(vx[0:16, :], ones16,
                     vc_row[0:1, half * SEG:(half + 1) * SEG],
                     start=True, stop=True, skip_group_check=True)
                  vxv = vx[0:16, :].rearrange("p (j t d) -> p j t d",
                                              t=2, d=DH)
                  nc.vector.tensor_copy(
                      out=vP2[0:NPM, half * 4:(half + 1) * 4, 0:DH],
                      in_=vxv[:, :, 0, :])
                  nc.vector.tensor_copy(
                      out=vP2[PFX:PFX + NPM, half * 4:(half + 1) * 4, 0:DH],
                      in_=vxv[:, :, 1, :])

              # ---------------- attention heads ----------------
              # Head PAIRS (2j, 2j+1) live at partition bases 0/64 of chunk j,
              # so their K=64 score matmuls run CONCURRENTLY on disjoint PE
              # row-groups. Software-pipelined: pair j+1's scores are emitted
              # before pair j's PV so PE never stalls on ACT's exps. The
              # softmax denominator rides as a ones-column of V (row DH of
              # po); each head's denom row is TT-copied to SBUF and gathered
              # into one [16,512] PSUM tile by a K=1 matmul, giving a single
              # batched reciprocal for all heads.
              dn_ps = dnp.tile([16, SEG], f32, tag="dn", name="dn_ps")

              def attn_scores(j):
                  eS = [[], []]
                  for c in range(2):
                      w = SEG - P * c
                      for hf in range(2):
                          qr = DH * hf
                          sc = psc_t()
                          mm(sc[:, 0:w],
                             kT[qr:qr + DH, j, c * P:(c + 1) * P],
                             qT[qr:qr + DH, j, c * P:],
                             start=True, stop=True, skip_group_check=True)
                          et = epool.tile([P, w], bf, tag="e0")
                          nc.scalar.activation(et, sc[:, 0:w], AF.Exp,
                                               bias=zeroc)
                          nc.vector.tensor_mul(et[:, 0:P], et[:, 0:P], maskD)
                          eS[hf].append(et)
                  # chunks 2+3 share one PSUM tile -> one exp per head
                  sc23 = [None, None]
                  for hf in range(2):
                      qr = DH * hf
                      sc23[hf] = psc_t()
                      mm(sc23[hf][:, 0:2 * P],
                         kT[qr:qr + DH, j, 2 * P:3 * P],
                         qT[qr:qr + DH, j, 2 * P:],
                         start=True, stop=False, skip_group_check=True)
                  for hf in range(2):
                      qr = DH * hf
                      mm(sc23[hf][:, 2 * P:3 * P],
                         kT[qr:qr + DH, j, 3 * P:],
                         qT[qr:qr + DH, j, 3 * P:],
                         start=False, stop=True, skip_group_check=True)
                  for hf in range(2):
                      et23 = epool.tile([P, 3 * P], bf, tag="e2")
                      nc.scalar.activation(et23, sc23[hf][:, 0:3 * P], AF.Exp,
                                           bias=zeroc)
                      nc.vector.tensor_mul(et23[:, 0:P], et23[:, 0:P], maskD)
                      nc.vector.tensor_mul(et23[:, 2 * P:3 * P],
                                           et23[:, 2 * P:3 * P], maskD)
                      eS[hf].append(et23[:, 0:2 * P])
                      eS[hf].append(et23[:, 2 * P:3 * P])
                  # paired prefix scores: even head rows 0:32, odd rows 32:64
                  scpp = psc_t()
                  mm(scpp[0:PFX, :], kP[0:DH, 2 * j, :], qT[0:DH, j, :],
                     start=True, stop=True, skip_group_check=True)
                  mm(scpp[PFX:2 * PFX, :], kP[DH:P, 2 * j + 1, :],
                     qT[DH:P, j, :],
                     start=True, stop=True, skip_group_check=True)
                  ePj = eppool.tile([2 * PFX, SEG], bf, tag="eP")
                  nc.scalar.activation(ePj, scpp[0:2 * PFX, :], AF.Exp,
                                       bias=zeroc[0:2 * PFX])
                  return eS, ePj

              def attn_pv(j, eS, ePj):
                  for hf in range(2):
                      h = 2 * j + hf
                      qr = DH * hf
                      b32 = PFX * hf
                      po = pso_t()
                      for c in range(4):
                          mm(po[0:DHP, c * P:], vA[:, c, h, :], eS[hf][c],
                             start=(c == 0), stop=False,
                             skip_group_check=True)
                      mm(po[0:DHP, :], vP2[b32:b32 + PFX, j, :],
                         ePj[b32:b32 + PFX, :],
                         start=False, stop=True, skip_group_check=True)
                      # unnormalized evac + denom row to SBUF
                      nc.vector.tensor_mul(oA[qr:qr + DH, j, :],
                                           po[0:DH, :], onesDH)
                      dnr = rot.tile([1, SEG], bf, tag="dnr")
                      nc.vector.tensor_add(dnr, po[DH:DHP, :],
                                           zrow64[DH:DHP, :])
                      # gather into row h of the shared denom tile (K=1)
                      mm(dn_ps[:, :], selv[0:1, h, :], dnr,
                         start=(h == 0), stop=(h == HEADS - 1),
                         skip_group_check=True)

              pend = attn_scores(0)
              for j in range(1, KO):
                  cur = attn_scores(j)
                  attn_pv(j - 1, *pend)
                  pend = cur
              attn_pv(KO - 1, *pend)
              nc.vector.reciprocal(rcp, dn_ps)
              for h in range(HEADS):
                  ko_h, hf = h // 2, h % 2
                  qr = DH * hf
                  bc = pa_t()
                  mm(bc[qr:qr + DH, :], selb[:, h, :], rcp,
                     start=True, stop=True, skip_group_check=True)
                  nc.vector.tensor_mul(oA[qr:qr + DH, ko_h, :],
                                       oA[qr:qr + DH, ko_h, :],
                                       bc[qr:qr + DH, :])

              # ---------------- output projection + residual ----------------
              for m in range(KO):
                  if m % 4 == 0:
                      owt = wpool.tile([P, 4, KO, P], bf, tag="w")
                      dma(out=owt, in_=outw_d[:, m:m + 4])
                  ps = pa_t()
                  for k in range(KO):
                      mm(ps, owt[:, m % 4, k], oA[:, k, :],
                         start=(k == 0), stop=(k == KO - 1))
                  nc.vector.tensor_add(xT[:, m, :], ps, xT[:, m, :])  # x1

              # ---------------- FFN (fp8e4 DoubleRow, weights x16) ----------
              # xn8 lives in qT's bytes, u8 in kT/oA's bytes (bitcast views).
              qTb = qT.bitcast(f8)   # [P, KO, 1024]
              kTb = kT.bitcast(f8)
              oAb = oA.bitcast(f8)

              def xn8_slice(ko):
                  return qTb[:, ko // 2, (ko % 2) * SEG:(ko % 2 + 1) * SEG]

              def u8_slice(k):
                  if k < 16:
                      return kTb[:, k // 2, (k % 2) * SEG:(k % 2 + 1) * SEG]
                  kk = k - 16
                  return oAb[:, kk // 2, (kk % 2) * SEG:(kk % 2 + 1) * SEG]

              rmsnorm_into(xn8_slice, xT, fnw)  # xn1 in fp8

              for j in range(11):
                  wt1 = wpool.tile([P, 2, 2, 4, 2, P], f8, tag="w")
                  dma(out=wt1, in_=w1_d[:, 2 * j:2 * j + 2])
                  for i in range(2):
                      m = 2 * j + i
                      psa = pa_t()
                      psg = pa_t()
                      for kq in range(4):
                          rhs = qTb[:, kq, :].rearrange("p (t n) -> p t n", t=2)
                          mm(psa, wt1[:, i, 0, kq], rhs, perf_mode=DR,
                             start=(kq == 0), stop=(kq == 3))
                          mm(psg, wt1[:, i, 1, kq], rhs, perf_mode=DR,
                             start=(kq == 0), stop=(kq == 3))
                      silu = rot.tile([P, SEG], bf, tag="silu")
                      nc.scalar.activation(silu, psg, AF.Silu,
                                           bias=b1g[:, m:m + 1], scale=1.0 / 16)
                      nc.vector.scalar_tensor_tensor(
                          out=u8_slice(m), in0=psa, scalar=b1a[:, m:m + 1],
                          in1=silu, op0=OP.add, op1=OP.mult)

              for o in range(KO):
                  w2t = w2pool.tile([P, 11, 2, P], f8, tag="w2")
                  dma(out=w2t, in_=w2_d[o])
                  ps = pa_t()
                  for kp in range(11):
                      if kp < 8:
                          rhs = kTb[:, kp, :].rearrange("p (t n) -> p t n", t=2)
                      else:
                          rhs = oAb[:, kp - 8, :].rearrange("p (t n) -> p t n",
                                                            t=2)
                      mm(ps, w2t[:, kp], rhs, perf_mode=DR,
                         start=(kp == 0), stop=(kp == 10))
                  outT = rot.tile([P, SEG], f32, tag="outT")
                  # ps carries 256*(u@w2) + 256*b2 (rank-1 bias row in w2)
                  nc.vector.scalar_tensor_tensor(
                      out=outT, in0=ps, scalar=inv256c,
                      in1=xT[:, o, :], op0=OP.mult, op1=OP.add)
                  dma(out=yT_d[o * P:(o + 1) * P, :], in_=outT)

    for _rep in range(reps):
        _emit(nc)
    nc.compile()
    return nc


# ======================= host-side preparation =======================

def _prep_shared(inputs):
    import ml_dtypes
    f32 = np.float32
    bf = ml_dtypes.bfloat16
    qkv = np.asarray(inputs["to_qkv_w"], f32)
    shared = {}
    # q/k projection weights: [p_in, m, ko, p_out]
    shared["qkw"] = np.ascontiguousarray(
        qkv[:, :2048].reshape(KO, P, 16, P).transpose(1, 2, 0, 3)).astype(bf)
    # k/v full weights: [p_in, c, ko, out]
    shared["kvw"] = np.ascontiguousarray(
        np.stack([qkv[:, 1024:2048], qkv[:, 2048:3072]])
        .reshape(2, KO, P, DIM).transpose(2, 0, 1, 3)).astype(bf)
    # out projection: [p_in, m, k, p_out]
    shared["outw"] = np.ascontiguousarray(
        np.asarray(inputs["to_out_w"], f32)
        .reshape(KO, P, KO, P).transpose(1, 2, 0, 3)).astype(bf)
    f8 = np.dtype(ml_dtypes.float8_e4m3)
    w1 = np.asarray(inputs["ff_w1"], f32)
    w1a = np.zeros((DIM, DFFP), f32)
    w1g = np.zeros((DIM, DFFP), f32)
    w1a[:, :DFF] = w1[:, :DFF]
    w1g[:, :DFF] = w1[:, DFF:]
    # ffn w1 (fp8, x16): [p_in, m, s(a/g), kq, i, p_out]; contraction dim
    # = (2*kq+i)*128 + p_in for DoubleRow ko-chunk pairing
    shared["w1"] = np.ascontiguousarray(
        (np.stack([w1a, w1g]) * 16.0).reshape(2, 4, 2, P, MFF, P)
        .transpose(3, 4, 0, 1, 2, 5)).astype(f8)
    w2 = np.zeros((DFFP, DIM), f32)
    w2[:DFF] = np.asarray(inputs["ff_w2"], f32)
    w2 *= 16.0
    # rank-1 bias row: u8[2815]=128 (from b1a/b1g pad), so 128*(2*b2)=256*b2
    w2[DFFP - 1, :] = 2.0 * np.asarray(inputs["ff_b2"], f32)
    # ffn w2 (fp8): [o, p_in, kp, i, p_out]
    shared["w2"] = np.ascontiguousarray(
        w2.reshape(11, 2, P, KO, P).transpose(3, 2, 0, 1, 4)).astype(f8)
    # mem tokens w: [p_in, ko, out]
    shared["mtw"] = np.ascontiguousarray(
        np.asarray(inputs["to_mem_tokens_w"], f32)
        .reshape(KO, P, DIM).transpose(1, 0, 2)).astype(bf)
    pm = np.asarray(inputs["persist_mem"], f32)
    pmv = np.ones((HEADS, NPM, DHP), f32)
    pmv[:, :, :DH] = pm
    shared["pmv"] = pmv.astype(bf)
    shared["pmk"] = np.ascontiguousarray(pm.transpose(0, 2, 1)).astype(bf)
    sel = np.zeros((P, HEADS, HEADS), f32)
    sel[:, np.arange(HEADS), np.arange(HEADS)] = 1.0
    shared["selv"] = sel.astype(bf)
    selb = np.zeros((HEADS, HEADS, DH), f32)
    selb[np.arange(HEADS), np.arange(HEADS), :] = 1.0
    shared["selb"] = selb.astype(bf)
    b1 = np.asarray(inputs["ff_b1"], f32)
    b1a = np.zeros(DFFP, f32)
    b1g = np.zeros(DFFP, f32)
    b1a[:DFF] = b1[:DFF]
    b1g[:DFF] = b1[DFF:]
    b1a *= 16.0          # u8 carries 16*u; silu arg uses scale=1/16
    b1a[DFFP - 1] = 6.4  # pad slot: u8 = 6.4*silu(20) = 128 (bias row)
    b1g[DFFP - 1] = 20.0
    shared["b1a"] = np.ascontiguousarray(b1a.reshape(MFF, P).T)
    shared["b1g"] = np.ascontiguousarray(b1g.reshape(MFF, P).T)
    shared["anw"] = np.ascontiguousarray(
        np.asarray(inputs["attn_norm_w"], f32).reshape(KO, P).T)
    shared["fnw"] = np.ascontiguousarray(
        np.asarray(inputs["ff_norm_w"], f32).reshape(KO, P).T)
    shared["mpnw"] = np.ascontiguousarray(
        np.asarray(inputs["mem_pool_norm_w"], f32).reshape(1, DIM))
    rl = np.zeros((P, P), f32)
    ii = np.arange(0, P, 2)
    rl[ii + 1, ii] = f32(-1.0)
    rl[ii, ii + 1] = f32(1.0)
    shared["rmat"] = rl.astype(bf)
    shared["ones"] = np.ones((P, P), bf)
    shared["maskD"] = np.where(
        np.arange(P)[None, :] >= np.arange(P)[:, None], f32(1.0), f32(0.0)
    ).astype(bf)

    # rope tables, float32 math to match the reference
    pos = np.arange(N, dtype=f32)
    expo = (np.arange(0, DH, 2).astype(f32) / f32(DH)).astype(f32)
    inv = (f32(1.0) / np.power(f32(10000.0), expo)).astype(f32)
    ang = np.repeat(pos[:, None] * inv[None, :], 2, axis=1).astype(f32)
    cosf, sinf = np.cos(ang).astype(f32), np.sin(ang).astype(f32)
    scale = f32(DH ** -0.5)
    shared["_cos"], shared["_sin"], shared["_scale"] = cosf, sinf, scale
    return shared


def _prep_core(inputs, shared, b, s):
    import ml_dtypes
    f32 = np.float32
    bf = ml_dtypes.bfloat16
    x = np.asarray(inputs["x"], f32)
    mo = np.asarray(inputs["mem_out"], f32)
    cosf, sinf, scale = shared["_cos"], shared["_sin"], shared["_scale"]
    seg = slice(s * SEG, (s + 1) * SEG)
    ct = np.ascontiguousarray(np.tile(cosf[seg].T, (2, 1)))
    st = np.ascontiguousarray(np.tile(sinf[seg].T, (2, 1)))
    m = {k: v for k, v in shared.items() if not k.startswith("_")}
    m["xT"] = np.ascontiguousarray(
        x[b, seg].T.reshape(KO, P, SEG).transpose(1, 0, 2))
    m["mo"] = np.ascontiguousarray(
        mo[b].reshape(16, P, DIM).transpose(1, 0, 2)).astype(bf)
    m["cq"] = (ct * scale).astype(bf)
    m["sq"] = (st * scale).astype(bf)
    m["ck"] = ct.astype(bf)
    m["sk"] = st.astype(bf)
    return m


def _get_nc():
    if "nc" not in _CACHE:
        _CACHE["nc"] = build_nc()
    return _CACHE["nc"]


def kernel(**inputs) -> np.ndarray:
    nc = _get_nc()
    shared = _prep_shared(inputs)
    cores = [(b, s) for b in range(B) for s in range(4)]
    in_maps = [_prep_core(inputs, shared, b, s) for b, s in cores]
    from concourse import bass_utils
    import os
    res = bass_utils.run_bass_kernel_spmd(
        nc, in_maps, core_ids=list(range(NCORES)),
        trace=bool(os.environ.get("MAC_TRACE")))
    _CACHE["last_results"] = res
    out = np.empty((B, N, DIM), np.float32)
    for i, (b, s) in enumerate(cores):
        out[b, s * SEG:(s + 1) * SEG, :] = res.results[i]["yT"].T
    return out
